# revision 15
# baseline (speedup 1.0000x reference)
"""Trainium2 Bass kernel for nn_AdditiveAttention (Bahdanau additive attention).

Shapes (hardcoded): B=16, Q=128, K=512, H=256, D=256, fp32 I/O.

  qh = q @ Wq.T + bq                       (B,Q,H)
  kh = k @ Wk.T + bk                       (B,K,H)
  scores[b,i,j] = Wv . tanh(qh[b,i,:] + kh[b,j,:])   (+bv, dropped: softmax-shift-invariant)
  masked softmax over j (j < valid_lens[b]), out = attn @ v   (B,Q,D)

Strategy: data-parallel over batch, 2 batches per core on 8 cores (one SPMD
NEFF). Host prep: shard + fp16-cast + transpose q/k (e-major), one-hot-expand
Wv into M=32 col-tiled weight tiles. Per core:
  stage 1 (per batch): project to qhT (h-part, q-free, f32) and
           khT (h-part, k-free, fp16) with bias folded in via ACT.
  stage 2 (batches interleaved): DVE tensor_scalar builds
           T[:, q*512:(q+1)*512] = khT + qhT[:, q] (fp16, 4x mode); ACT tanh
           per (q-group, k-block) into dense Tout tiles; PE reduces over h
           with the one-hot Wv weights (M=32, col-tiled) accumulating score
           rows into PSUM. Mask rows (built from valid_lens on device) are
           added via a K=1 matmul that also opens the PSUM accumulation group.
  stage 3: masked softmax (k on free axis; exp's accum_out gives row sums),
           PE-transpose of attn, attn @ v, scale by 1/denom, DMA out.

k-blocks that are entirely masked (128*c >= valid_len) skip their tanh +
score matmuls at runtime via tc.If on registers loaded from valid_lens
(condition registers only on ACT/PE so other engines never branch).
"""

import numpy as np

B, Q, K, H, D = 16, 128, 512, 256, 256
NB = 2          # batches per core
NCORES = 8
NEG = -1.0e6
QG = 32         # queries per group (matmul col-tile M)
NKB = 4         # k blocks of 128
NHB = 2         # h blocks of 128
NEB = 2         # e (embedding) blocks of 128


def slot_kbs_for(valid_lens):
    vl = np.asarray(valid_lens).astype(np.int64)
    order = np.argsort(-vl, kind="stable")
    kb = np.minimum((vl + 127) // 128, NKB)
    return (int(kb[order[:NCORES]].max()), int(kb[order[NCORES:]].max()))


def build_program(slot_kbs=(NKB, NKB)):
    import concourse.bass as bass
    import concourse.mybir as mybir
    from concourse import bacc, masks
    from concourse.ordered_set import OrderedSet
    from concourse.tile import TileContext

    f32 = mybir.dt.float32
    f16 = mybir.dt.float16
    bf16 = mybir.dt.bfloat16
    i32 = mybir.dt.int32
    AF = mybir.ActivationFunctionType
    ALU = mybir.AluOpType
    X = mybir.AxisListType.X

    nc = bacc.Bacc("TRN2", target_bir_lowering=False, debug=False)

    qtin = nc.dram_tensor("qtin", [NB, H, Q], f16, kind="ExternalInput")   # (e, q)
    ktin = nc.dram_tensor("ktin", [NB, H, K], f16, kind="ExternalInput")   # (e, k)
    vtin = nc.dram_tensor("vtin", [NB, K, D], f16, kind="ExternalInput")
    lens_f = nc.dram_tensor("lens_f", [1, NB], f32, kind="ExternalInput")
    lens_i = nc.dram_tensor("lens_i", [1, NB], i32, kind="ExternalInput")
    wqt = nc.dram_tensor("wqt", [H, H], f16, kind="ExternalInput")   # Wq.T (e,h)
    wkt = nc.dram_tensor("wkt", [H, H], f16, kind="ExternalInput")   # Wk.T (e,h)
    bqc = nc.dram_tensor("bqc", [128, NHB], f32, kind="ExternalInput")
    bkc = nc.dram_tensor("bkc", [128, NHB], f32, kind="ExternalInput")
    wvx = nc.dram_tensor("wvx", [128, NHB * QG * QG], f16, kind="ExternalInput")
    iota = nc.dram_tensor("iota", [1, K], f32, kind="ExternalInput")
    out = nc.dram_tensor("out", [NB, Q, D], f32, kind="ExternalOutput")

    with TileContext(nc) as tc:
        with (
            tc.tile_pool(name="const", bufs=1) as cpool,
            tc.tile_pool(name="perb", bufs=2) as bpool,
            tc.tile_pool(name="tt", bufs=3) as ttpool,
            tc.tile_pool(name="tout", bufs=4) as topool,
            tc.tile_pool(name="att", bufs=2) as atpool,
            tc.tile_pool(name="stat", bufs=2) as stpool,
            tc.tile_pool(name="ps_tp", bufs=2, space="PSUM") as ps_tp,
            tc.tile_pool(name="ps_mm", bufs=2, space="PSUM") as ps_mm,
            tc.tile_pool(name="ps_sc", bufs=2, space="PSUM") as ps_sc,
            tc.tile_pool(name="ps_out", bufs=2, space="PSUM") as ps_out,
        ):
            # ---------------- constants / weights ----------------
            ident16 = cpool.tile([128, 128], f16)
            masks.make_identity(nc, ident16[:])
            ones_row = cpool.tile([1, 128], bf16)
            nc.vector.memset(ones_row[:], 1.0)
            zeros_row = cpool.tile([1, K], bf16)
            nc.vector.memset(zeros_row[:], 0.0)

            iota_sb = cpool.tile([1, K], f32)
            nc.sync.dma_start(iota_sb[:], iota[:])
            lensf_sb = cpool.tile([1, NB], f32)
            nc.sync.dma_start(lensf_sb[:], lens_f[:])
            lensi_sb = cpool.tile([1, NB], i32)
            nc.sync.dma_start(lensi_sb[:], lens_i[:])

            bq_sb = cpool.tile([128, NHB], f32)
            nc.sync.dma_start(bq_sb[:], bqc[:])
            bk_sb = cpool.tile([128, NHB], f32)
            nc.sync.dma_start(bk_sb[:], bkc[:])
            wvx_sb = cpool.tile([128, NHB * QG * QG], f16)
            nc.sync.dma_start(wvx_sb[:], wvx[:])

            # projection weights, (e-part, [eb, h]) fp16
            wqt_sb = cpool.tile([128, NEB * H], f16)
            nc.sync.dma_start(
                wqt_sb[:], wqt.rearrange("(e p) h -> p e h", p=128)
            )
            wkt_sb = cpool.tile([128, NEB * H], f16)
            nc.sync.dma_start(
                wkt_sb[:], wkt.rearrange("(e p) h -> p e h", p=128)
            )

            # additive mask rows per batch: 0 where j < len, NEG where j >= len
            mask_rows = cpool.tile([1, NB * K], bf16)
            for b in range(NB):
                nc.vector.tensor_scalar(
                    mask_rows[:, b * K : (b + 1) * K],
                    iota_sb[:],
                    lensf_sb[:, b : b + 1],
                    NEG,
                    op0=ALU.is_ge,
                    op1=ALU.mult,
                )

            # ---------------- stage 1: loads + projections ----------------
            qT16, kT16, v16, khT, qhT, ps_scores = {}, {}, {}, {}, {}, {}
            for b in range(NB):
                qT16[b] = bpool.tile([128, NEB * Q], f16, tag="qT", name=f"qT{b}")
                nc.sync.dma_start(
                    qT16[b][:], qtin[b].rearrange("(e p) q -> p e q", p=128)
                )
                Wb = slot_kbs[b] * 128
                kT16[b] = bpool.tile([128, NEB * Wb], f16, tag="kT", name=f"kT{b}")
                nc.sync.dma_start(
                    kT16[b][:],
                    ktin[b].rearrange("(e p) k -> p e k", p=128)[:, :, :Wb],
                )
                v16[b] = bpool.tile(
                    [128, slot_kbs[b] * D], f16, tag="v", name=f"v16_{b}"
                )
                nc.sync.dma_start(
                    v16[b][:],
                    vtin[b].rearrange("(c p) d -> p c d", p=128)[
                        :, : slot_kbs[b], :
                    ],
                )

                khT[b] = bpool.tile(
                    [128, NHB * Wb], f16, tag="khT", name=f"khT{b}"
                )
                qhT[b] = bpool.tile([128, NHB * 128], f32, tag="qhT", name=f"qhT{b}")
                for hb in range(NHB):
                    pk = ps_mm.tile([128, K], f32, tag="mm")
                    for eb in range(NEB):
                        nc.tensor.matmul(
                            pk[:, :Wb],
                            wkt_sb[:, eb * H + hb * 128 : eb * H + (hb + 1) * 128],
                            kT16[b][:, eb * Wb : (eb + 1) * Wb],
                            start=(eb == 0),
                            stop=(eb == NEB - 1),
                        )
                    nc.scalar.activation(
                        khT[b][:, hb * Wb : (hb + 1) * Wb],
                        pk[:, :Wb],
                        AF.Identity,
                        bias=bk_sb[:, hb : hb + 1],
                    )
                    pq = ps_mm.tile([128, K], f32, tag="mm")
                    for eb in range(NEB):
                        nc.tensor.matmul(
                            pq[:, :128],
                            wqt_sb[:, eb * H + hb * 128 : eb * H + (hb + 1) * 128],
                            qT16[b][:, eb * Q : (eb + 1) * Q],
                            start=(eb == 0),
                            stop=(eb == NEB - 1),
                        )
                    nc.scalar.activation(
                        qhT[b][:, hb * 128 : (hb + 1) * 128],
                        pq[:, :128],
                        AF.Identity,
                        bias=bq_sb[:, hb : hb + 1],
                    )

                ps_scores[b] = ps_sc.tile(
                    [128, Wb], f32, tag="sc", name=f"scores{b}"
                )
                # opens the accumulation group over the whole tile + adds mask
                nc.tensor.matmul(
                    ps_scores[b][:],
                    ones_row[:],
                    mask_rows[:, b * K : b * K + Wb],
                    start=True,
                    stop=False,
                )

            # ------- stage 2: tanh features + score reduction (interleaved) --
            def act_work(b, hb, g, c, Tt, To):
                Tt3 = Tt[:].rearrange("p (q k) -> p q k", k=slot_kbs[b] * 128)
                To3 = To[:].rearrange("p (q k) -> p q k", k=128)
                nc.scalar.activation(
                    To3,
                    Tt3[:, :, c * 128 : (c + 1) * 128],
                    AF.Tanh,
                )

            def pe_work(b, hb, g, c, To):
                for r in range(QG):
                    qblk = g * QG
                    wcol = (hb * QG + r) * QG
                    nc.tensor.matmul(
                        ps_scores[b][qblk : qblk + QG, c * 128 : (c + 1) * 128],
                        wvx_sb[:, wcol : wcol + QG],
                        To[:, r * 128 : (r + 1) * 128],
                        start=False,
                        stop=False,
                        skip_group_check=True,
                        tile_position=(0, qblk),
                    )

            for hb in range(NHB):
                for g in range(Q // QG):
                    for b in range(NB):
                        Wb = slot_kbs[b] * 128
                        Tt = ttpool.tile([128, QG * Wb], f16, tag="Tt")
                        for r in range(QG):
                            qq = g * QG + r
                            nc.vector.tensor_scalar_add(
                                Tt[:, r * Wb : (r + 1) * Wb],
                                khT[b][:, hb * Wb : (hb + 1) * Wb],
                                qhT[b][:, hb * 128 + qq : hb * 128 + qq + 1],
                            )
                        for c in range(slot_kbs[b]):
                            To = topool.tile([128, QG * 128], f16, tag="To")
                            act_work(b, hb, g, c, Tt, To)
                            pe_work(b, hb, g, c, To)

            # ---------------- stage 3: softmax + attn @ v ----------------
            for b in range(NB):
                Wb = slot_kbs[b] * 128
                # closes the accumulation group over the whole tile (adds 0)
                nc.tensor.matmul(
                    ps_scores[b][:], ones_row[:], zeros_row[:, :Wb],
                    start=False, stop=True,
                )
                nmax = stpool.tile([128, 1], f32, tag="nmax")
                nc.vector.tensor_reduce(
                    nmax[:], ps_scores[b][:], axis=X, op=ALU.max, negate=True
                )
                p_sb = bpool.tile([128, Wb], f16, tag="p")
                denom = stpool.tile([128, 1], f32, tag="den")
                nc.scalar.activation(
                    p_sb[:], ps_scores[b][:], AF.Exp, bias=nmax[:],
                    accum_out=denom[:],
                )
                recip = stpool.tile([128, 1], f32, tag="recip")
                nc.vector.reciprocal(recip[:], denom[:])

                po = ps_out.tile([128, D], f32, tag="po")
                for c in range(slot_kbs[b]):
                    ptp = ps_tp.tile([128, 128], f16, tag="tp")
                    nc.tensor.transpose(
                        ptp[:], p_sb[:, c * 128 : (c + 1) * 128], ident16[:]
                    )
                    aT = atpool.tile([128, 128], f16, tag="aT")
                    nc.vector.tensor_copy(aT[:], ptp[:])
                    nc.tensor.matmul(
                        po[:],
                        aT[:],
                        v16[b][:, c * D : (c + 1) * D],
                        start=(c == 0),
                        stop=(c == slot_kbs[b] - 1),
                    )

                out_sb = bpool.tile([128, D], f32, tag="osb")
                nc.vector.tensor_scalar_mul(out_sb[:], po[:], recip[:])
                nc.sync.dma_start(out[b], out_sb[:])

    nc.compile()
    return nc


def _in_maps(q, k, v, valid_lens, Wq, bq, Wk, bk, Wv):
    """Shard batches to cores (len-balanced pairing) + replicated weights."""
    vl = np.asarray(valid_lens).astype(np.int64)
    order = np.argsort(-vl, kind="stable")
    assign = [(int(order[i]), int(order[2 * NCORES - 1 - i])) for i in range(NCORES)]

    wvx = np.zeros((128, NHB * QG * QG), np.float16)
    Wv32 = np.asarray(Wv, np.float32)
    for hb in range(NHB):
        for r in range(QG):
            wvx[:, (hb * QG + r) * QG + r] = Wv32[hb * 128 : (hb + 1) * 128]

    shared = {
        "wqt": np.ascontiguousarray(np.asarray(Wq, np.float32).T).astype(np.float16),
        "wkt": np.ascontiguousarray(np.asarray(Wk, np.float32).T).astype(np.float16),
        "bqc": np.ascontiguousarray(np.asarray(bq, np.float32).reshape(NHB, 128).T),
        "bkc": np.ascontiguousarray(np.asarray(bk, np.float32).reshape(NHB, 128).T),
        "wvx": wvx,
        "iota": np.arange(K, dtype=np.float32).reshape(1, K),
    }
    q = np.asarray(q, np.float32)
    k = np.asarray(k, np.float32)
    v = np.asarray(v, np.float32)
    maps = []
    for a, b2 in assign:
        idx = [a, b2]
        maps.append(
            dict(
                shared,
                qtin=np.ascontiguousarray(
                    q[idx].transpose(0, 2, 1).astype(np.float16)
                ),
                ktin=np.ascontiguousarray(
                    k[idx].transpose(0, 2, 1).astype(np.float16)
                ),
                vtin=np.ascontiguousarray(v[idx].astype(np.float16)),
                lens_f=vl[idx].astype(np.float32).reshape(1, NB),
                lens_i=vl[idx].astype(np.int32).reshape(1, NB),
            )
        )
    return maps, assign


_cached_nc = {}


def kernel(q, k, v, valid_lens, Wq, bq, Wk, bk, Wv, bv, trace=False):
    from concourse.bass_utils import run_bass_kernel_spmd

    slot_kbs = slot_kbs_for(valid_lens)
    if slot_kbs not in _cached_nc:
        _cached_nc[slot_kbs] = build_program(slot_kbs=slot_kbs)
    nc = _cached_nc[slot_kbs]
    in_maps, assign = _in_maps(q, k, v, valid_lens, Wq, bq, Wk, bk, Wv)
    res = run_bass_kernel_spmd(nc, in_maps, core_ids=list(range(NCORES)), trace=trace)
    full = np.empty((B, Q, D), np.float32)
    for c, (a, b2) in enumerate(assign):
        full[a] = res.results[c]["out"][0]
        full[b2] = res.results[c]["out"][1]
    kernel.last_result = res
    return full
